# revision 24
# baseline (speedup 1.0000x reference)
"""Trainium2 Bass kernel for nn_CaptioningRNN (attention LSTM over T=64).

Data-parallel over the batch: N=256 samples split across 8 NeuronCores
(32 samples/core), weights replicated, no collectives.

Per-core algorithm (all matmuls bf16 on the TensorEngine, state in f32):
  1. xproj: xpT = (x @ Wx + b) computed transposed via Wx-stationary
     matmuls into a DRAM scratch.  Time is split into 8 blocks of 8
     steps; block 0 runs up front, blocks 1-7 are emitted as PE filler
     inside the recurrence (4 chunks per step, steps 0-55) at the three
     points where the serial softmax/cast/cell chains would otherwise
     idle the PE and let HAM re-throttle the clock.
  2. P phase: P[n, k, :] = A[n, :, k] @ Wattn precomputed once (the
     attention context contribution to the gates becomes a w-weighted
     sum of P rows).  h0 = c0 = mean_k(A) from an f32 copy of A.
  3. Recurrence (64 steps):
     - scores via hT-chunk matmuls against a permuted A (cross-sample
       products in PSUM, diagonal extracted with a mask+reduce on DVE)
     - softmax on [32,16] via the sigmoid table (no max-subtract: the
       scores are O(1) by construction), e^s = y/(1-y)
     - w compacted/expanded on PE to a block-diagonal stationary
     - gates = h @ Wh + sum_k w_k P_k accumulated into ONE [128,1024]
       PSUM strip pair using 4-way tensor-engine column tiling
     - one cast, 8 PE transposes into one bf16 PSUM bank, one xproj
       add, strided activations, cell math on [128,256] views
  4. Output written transposed [t, h, n]; host reassembles to (N, T, H).
"""

from contextlib import ExitStack

import numpy as np
import ml_dtypes

import concourse.bacc as bacc
import concourse.mybir as mybir
from concourse import bass_utils
from concourse.tile import TileContext, add_dep_helper

F32, BF16 = mybir.dt.float32, mybir.dt.bfloat16
AF = mybir.ActivationFunctionType
ALU = mybir.AluOpType
AX = mybir.AxisListType
BF = ml_dtypes.bfloat16

N, T, D, H = 256, 64, 1024, 1024
NCORES = 8
NL = N // NCORES          # 32 samples per core
HC = 8                    # 128-row chunks of D/H
G, GS = 4, 8              # sample groups of 8 (for the (k, n_g) 128-partition layout)
H4 = 4 * H                # 4096 gate columns
TB = 8                    # xproj time-block length (steps per chunk)

_built = None


def _consts():
    # E16[k', 8k + n] = (k' == k): one-hot expansion of wT rows onto the
    # (k-major, n_g-minor) 128-partition layout.
    e16 = np.zeros((16, 128), dtype=BF)
    for k in range(16):
        e16[k, 8 * k : 8 * k + 8] = 1
    # M32R[p, 128 g + 32 rep + m] = (m % 8 == p % 8) & (m // 8 == g):
    # block-diagonal mask producing masked_g = w[m, k(p)] only for group-g
    # samples, replicated 4x for the column-tiled matmuls.
    p = np.arange(128)[:, None]
    m = np.arange(32)[None, :]
    m32r = np.zeros((128, 512), dtype=BF)
    for g in range(4):
        blk = ((m % 8 == p % 8) & (m // 8 == g)).astype(BF)
        for rep in range(4):
            m32r[:, 128 * g + 32 * rep : 128 * g + 32 * rep + 32] = blk
    # Mdiag8[32 g + m, 8 k + n] = (m == 8 g + n) / 32: extracts the
    # group-local diagonal of the score products (stationary = all 32
    # samples, moving = group-g A columns) and applies the 1/sqrt(H) scale.
    md8 = np.zeros((128, 128), dtype=np.float32)
    for g in range(4):
        for n in range(8):
            for k in range(16):
                md8[32 * g + 8 * g + n, 8 * k + n] = 1.0 / 32.0
    # selT[32 g + (8 g + n), 8 g + n] = 1: compacts the block-diagonal w
    # layout to wT[k, n] via a single PE matmul (stationary = w2).
    sel = np.zeros((128, 32), dtype=BF)
    for g in range(4):
        for n in range(8):
            sel[32 * g + 8 * g + n, 8 * g + n] = 1
    return e16, m32r, md8, sel


def _build_nc(t_steps=T):
    nc = bacc.Bacc(trn_type="TRN2", target_bir_lowering=False, debug=False)

    ap_xT = nc.dram_tensor("xT", [D, T * NL], BF16, kind="ExternalInput").ap()
    ap_Asc = nc.dram_tensor("Asc", [H, 512], BF16, kind="ExternalInput").ap()
    ap_Asc32 = nc.dram_tensor("Asc32", [H, 512], F32, kind="ExternalInput").ap()
    ap_Wx = nc.dram_tensor("Wx", [D, H4], BF16, kind="ExternalInput").ap()
    ap_Wh = nc.dram_tensor("Wh", [H, H4], BF16, kind="ExternalInput").ap()
    ap_Wattn = nc.dram_tensor("Wattn", [H, H4], BF16, kind="ExternalInput").ap()
    ap_bT = nc.dram_tensor("bT", [128, 32], F32, kind="ExternalInput").ap()
    outT = nc.dram_tensor("outT", [T, H, NL], F32, kind="ExternalOutput").ap()
    # xps[r, t, q, j, p, n] = xproj[t][n, j*1024 + r*512 + q*128 + p]
    # (t outermost-after-r so the per-step load is one contiguous 64 KiB
    # block per r; the chunk stores scatter over t but have deep slack)
    xps = nc.dram_tensor("xps", [2, T, 4, 4, 128, NL], BF16, kind="Internal").ap()

    e16_np, m32r_np, md8_np, sel_np = _consts()
    eye_d = nc.inline_tensor(np.eye(128, dtype=BF), "c_eye")
    e16_d = nc.inline_tensor(e16_np, "c_e16")
    m32r_d = nc.inline_tensor(m32r_np, "c_m32r")
    md8_d = nc.inline_tensor(md8_np, "c_mdiag8")
    sel_d = nc.inline_tensor(sel_np, "c_selT")

    with TileContext(nc) as tc:
        with tc.tile_pool(name="pers", bufs=1) as pers:
            Wh_sb = pers.tile([128, HC * H4], BF16, tag="Wh")
            Asc_sb = pers.tile([128, HC * 512], BF16, tag="Asc")
            P_sb = pers.tile([128, G * H4], BF16, tag="P")
            uTh = pers.tile([128, HC * 32], BF16, tag="uTh")
            cT = pers.tile([128, 256], F32, tag="cT")
            eye = pers.tile([128, 128], BF16, tag="eye")
            E16 = pers.tile([16, 128], BF16, tag="E16")
            M32R = pers.tile([128, 512], BF16, tag="M32R")
            Mdiag8 = pers.tile([128, 128], F32, tag="Mdiag8")
            selT = pers.tile([128, 32], BF16, tag="selT")
            b_sb = pers.tile([128, 32], F32, tag="bT")
            Ag = pers.tile([128, G * HC * 128], BF16, tag="Ag")

            nc.sync.dma_start(eye[:], eye_d.ap()[:])
            nc.sync.dma_start(E16[:], e16_d.ap()[:])
            nc.sync.dma_start(M32R[:], m32r_d.ap()[:])
            nc.sync.dma_start(Mdiag8[:], md8_d.ap()[:])
            nc.sync.dma_start(selT[:], sel_d.ap()[:])
            nc.sync.dma_start(b_sb[:], ap_bT[:])
            nc.sync.dma_start(
                Wh_sb[:].rearrange("p (c x) -> p c x", c=HC),
                ap_Wh.rearrange("(c p) x -> p c x", p=128),
            )
            nc.sync.dma_start(
                Asc_sb[:].rearrange("p (c x) -> p c x", c=HC),
                ap_Asc.rearrange("(c p) x -> p c x", p=128),
            )

            # ---------------- phase A: xproj -> DRAM scratch ----------------
            # Pools stay open through the recurrence so block>=1 chunks can
            # be interleaved between steps (fills PE-idle gaps, keeps HAM
            # warm).
            _ax = ExitStack()
            phx1 = _ax.enter_context(tc.tile_pool(name="phx1", bufs=1))
            phxW = _ax.enter_context(tc.tile_pool(name="phxW", bufs=8))
            phxS = _ax.enter_context(tc.tile_pool(name="phxS", bufs=6))
            psX = _ax.enter_context(tc.tile_pool(name="psX", bufs=2, space="PSUM"))
            xT_sb = phx1.tile([128, HC * T * NL], BF16, tag="xTsb")
            # split the 4 MB load so the first chunks can start early
            for c in range(HC):
                nc.sync.dma_start(
                    xT_sb[:, c * T * NL : (c + 1) * T * NL],
                    ap_xT.rearrange("(c p) x -> p c x", p=128)[:, c],
                )

            nchunk = [0]
            # xps store DMA instructions per time-block, so the per-step
            # xpt loads can take an explicit dependency on them (the DRAM
            # scratch is not covered by tile overlap tracking)
            xps_stores = [[] for _ in range(T // TB)]

            def xproj_mm(W, tb):
                j, r, q = W // 8, (W % 8) // 4, W % 4
                t0, t1 = TB * tb, TB * (tb + 1)
                nt = (t1 - t0) * NL
                Wxb = phxW.tile(
                    [128, HC * 128], BF16, tag="Wxb", name=f"Wxb_{W}_{tb}"
                )
                nc.sync.dma_start(
                    Wxb[:].rearrange("p (c x) -> p c x", c=HC),
                    ap_Wx.rearrange("(c p) x -> p c x", p=128)[
                        :, :, 128 * W : 128 * (W + 1)
                    ],
                )
                psx = psX.tile([128, nt], F32, tag="psx", name=f"psx_{W}_{tb}")
                for c in range(HC):
                    nc.tensor.matmul(
                        psx[:],
                        Wxb[:, c * 128 : (c + 1) * 128],
                        xT_sb[:, c * T * NL + NL * t0 : c * T * NL + NL * t1],
                        start=(c == 0),
                        stop=(c == HC - 1),
                    )
                return psx

            def xproj_evac(W, tb, psx):
                j, r, q = W // 8, (W % 8) // 4, W % 4
                t0, t1 = TB * tb, TB * (tb + 1)
                nt = (t1 - t0) * NL
                ci = nchunk[0]
                nchunk[0] += 1
                sxp = phxS.tile([128, nt], BF16, tag="sxp", name=f"sxp_{W}_{tb}")
                # alternate the PSUM evacuation engine so chunk fills don't
                # serialize behind the step's ACT/DVE chain work
                if ci % 2 == 0:
                    nc.scalar.add(sxp[:], psx[:], b_sb[:, W : W + 1])
                else:
                    nc.vector.tensor_scalar_add(sxp[:], psx[:], b_sb[:, W : W + 1])
                # store on the ACT HWDGE queue so the Wxb prefetch stream
                # (sync queue) is never stuck behind these scatters
                st = nc.scalar.dma_start(
                    xps[r, t0:t1, q, j].transpose([1, 0, 2]),
                    sxp[:].rearrange("p (t n) -> p t n", t=t1 - t0),
                )
                xps_stores[tb].append(st.ins)

            def xproj_chunk(W, tb):
                xproj_evac(W, tb, xproj_mm(W, tb))

            # block 0 up front (also serves as the HAM warm-up ramp)
            for W in range(32):
                xproj_chunk(W, 0)

            # ------------- phase B: P precompute + h0/c0 init -------------
            with tc.tile_pool(name="php1", bufs=1) as php1, \
                 tc.tile_pool(name="php", bufs=3) as php, \
                 tc.tile_pool(name="psP", bufs=2, space="PSUM") as psP:
                A32 = php1.tile([128, HC * 512], F32, tag="A32")
                nc.sync.dma_start(
                    A32[:].rearrange("p (c x) -> p c x", c=HC),
                    ap_Asc32.rearrange("(c p) x -> p c x", p=128),
                )
                for c in range(HC):
                    h0s = php.tile([128, 32], F32, tag="h0s")
                    nc.vector.tensor_reduce(
                        h0s[:],
                        A32[:, c * 512 : (c + 1) * 512].rearrange(
                            "p (k n) -> p n k", k=16
                        ),
                        axis=AX.X,
                        op=ALU.add,
                    )
                    nc.vector.tensor_scalar_mul(
                        cT[:, 32 * c : 32 * (c + 1)], h0s[:], 1.0 / 16.0
                    )
                    nc.vector.tensor_copy(
                        uTh[:, 32 * c : 32 * (c + 1)],
                        cT[:, 32 * c : 32 * (c + 1)],
                    )
                # contiguous staging of the group-selected A columns so the
                # matmul stationary operand has a single free dim
                for g in range(G):
                    for c in range(HC):
                        nc.vector.tensor_copy(
                            Ag[:, (g * HC + c) * 128 : (g * HC + c) * 128 + 128],
                            Asc_sb[:, c * 512 : (c + 1) * 512].rearrange(
                                "p (k n) -> p k n", k=16
                            )[:, :, GS * g : GS * (g + 1)],
                        )
                for blk in range(8):
                    Wab = php.tile([128, HC * 512], BF16, tag="Wab")
                    nc.sync.dma_start(
                        Wab[:].rearrange("p (c x) -> p c x", c=HC),
                        ap_Wattn.rearrange("(c p) x -> p c x", p=128)[
                            :, :, 512 * blk : 512 * (blk + 1)
                        ],
                    )
                    for g in range(G):
                        psp = psP.tile([128, 512], F32, tag="psp")
                        for c in range(HC):
                            nc.tensor.matmul(
                                psp[:],
                                Ag[:, (g * HC + c) * 128 : (g * HC + c) * 128 + 128],
                                Wab[:, c * 512 : (c + 1) * 512],
                                start=(c == 0),
                                stop=(c == HC - 1),
                            )
                        nc.vector.tensor_copy(
                            P_sb[:, g * H4 + 512 * blk : g * H4 + 512 * (blk + 1)],
                            psp[:],
                        )

            # ---------------------- phase C: recurrence ----------------------
            with tc.tile_pool(name="wrk", bufs=2) as wrk, \
                 tc.tile_pool(name="psc", bufs=1, space="PSUM") as psc_pool, \
                 tc.tile_pool(name="pwx", bufs=1, space="PSUM") as pwx_pool, \
                 tc.tile_pool(name="pstr", bufs=1, space="PSUM") as pstr_pool, \
                 tc.tile_pool(name="paT", bufs=1, space="PSUM") as paT_pool:
                # deferred xproj chunks in deadline order: block tb is
                # consumed during steps [8*(tb-1), 8*tb) and must land
                # before step 8*tb reads it (Tile semaphores enforce it).
                chunks = [
                    (W, tb)
                    for tb in range(1, T // TB)
                    if TB * tb < t_steps
                    for W in range(32)
                ]
                ci = 0
                mv = lambda ap: ap.rearrange("p (m x) -> p m x", m=8)
                mn = lambda ap: ap.rearrange("p (m n) -> p m n", m=8)
                for t in range(t_steps):
                    # chunk matmuls are emitted in the tail stall windows;
                    # their PSUM evacuations are deferred to the end of the
                    # step so they land on ACT/DVE idle time during the next
                    # step's scores/gates instead of inside the serial chain
                    pend = []

                    def fill(k):
                        nonlocal ci
                        want = min(len(chunks), 4 * (t + 1) + k)
                        while ci < want:
                            W_, tb_ = chunks[ci]
                            pend.append((W_, tb_, xproj_mm(W_, tb_)))
                            ci += 1

                    # prefetched xproj slice for this step, laid out
                    # (r, q, j, n) to match the transposed strip
                    xptf = wrk.tile([128, 1024], BF16, tag="xpt", name=f"xpt_{t}")
                    for r in range(2):
                        ld = nc.gpsimd.dma_start(
                            xptf[:, 512 * r : 512 * (r + 1)].rearrange(
                                "p (c n) -> p c n", c=16
                            ),
                            xps[r, t].rearrange("q j p n -> p (q j) n"),
                        )
                        for st_ins in xps_stores[t // TB]:
                            add_dep_helper(
                                ld.ins, st_ins, reason="xps block store->load"
                            )

                    # -- scores: per-group (8-sample) products against Ag with
                    # 4-way col tiling, group-local diag extract, softmax
                    pscg = psc_pool.tile([128, 128], F32, tag="psc")
                    for c in range(HC):
                        for g in range(G):
                            nc.tensor.matmul(
                                pscg[32 * g : 32 * (g + 1), :],
                                uTh[:, c * 32 : (c + 1) * 32],
                                Ag[:, (g * HC + c) * 128 : (g * HC + c + 1) * 128],
                                start=(c == 0),
                                stop=(c == HC - 1),
                                skip_group_check=True,
                                tile_position=(0, 32 * g),
                            )
                    scm = wrk.tile([128, 128], F32, tag="scm")
                    nc.vector.tensor_mul(scm[:], pscg[:], Mdiag8[:])
                    scores = wrk.tile([128, 16], F32, tag="scores")
                    nc.vector.tensor_reduce(
                        scores[:],
                        scm[:].rearrange("p (k n) -> p k n", k=16),
                        axis=AX.X,
                        op=ALU.add,
                    )
                    # softmax via the sigmoid table (keeps every ACT op in the
                    # sigmoid_and_others set -> one table load for the kernel):
                    # y = sigmoid(s), e^s = y / (1 - y); the scores are O(1)
                    # by construction so instead of a max-subtraction a clamp
                    # at 12 guards the y -> 1 division (a no-op in practice)
                    scl = wrk.tile([128, 16], F32, tag="scl")
                    nc.vector.tensor_scalar_min(scl[:], scores[:], 12.0)
                    ysig = wrk.tile([128, 16], F32, tag="ysig")
                    nc.scalar.activation(ysig[:], scl[:], AF.Sigmoid)

                    # warm-keepers: only for the filler-less tail steps
                    warm_on = ci >= len(chunks) and t < t_steps - 1

                    def warm(i, dep):
                        if warm_on:
                            nc.tensor.matmul(
                                pscg[0:16, 16 * i : 16 * (i + 1)],
                                dep,
                                dep,
                                start=True,
                                stop=True,
                            )

                    omy = wrk.tile([128, 16], F32, tag="omy")
                    nc.vector.tensor_scalar(
                        omy[:], ysig[:], -1.0, 1.0, ALU.mult, ALU.add
                    )
                    romy = wrk.tile([128, 16], F32, tag="romy")
                    nc.vector.reciprocal(romy[:], omy[:])
                    ex = wrk.tile([128, 16], F32, tag="ex")
                    esum = wrk.tile([128, 1], F32, tag="esum")
                    nc.vector.scalar_tensor_tensor(
                        ex[:], ysig[:], 1.0, romy[:], ALU.mult, ALU.mult,
                        accum_out=esum[:],
                    )
                    rcp = wrk.tile([128, 1], F32, tag="rcp")
                    nc.vector.reciprocal(rcp[:], esum[:])
                    w2 = wrk.tile([128, 16], BF16, tag="w2")
                    nc.vector.tensor_scalar_mul(w2[:], ex[:], rcp[:])

                    # -- gates: h @ Wh + sum_k w_k P_k into one column-tiled
                    # [128, 1024] strip pair (r = 512-col halves).  The Wh
                    # matmuls are emitted BEFORE the w-expand PE ops so the
                    # softmax chain hides behind ~4.4us of scores+Wh instead
                    # of head-of-line-blocking the PE queue at wTps.
                    strips = pstr_pool.tile([128, 1024], F32, tag="strips",
                                            name=f"strips_{t}")
                    for c in range(HC):
                        for r in range(2):
                            for j in range(4):
                                nc.tensor.matmul(
                                    strips[32 * j : 32 * (j + 1),
                                           512 * r : 512 * (r + 1)],
                                    uTh[:, c * 32 : (c + 1) * 32],
                                    Wh_sb[:, c * H4 + j * 1024 + r * 512 : c * H4 + j * 1024 + r * 512 + 512],
                                    start=(c == 0),
                                    stop=False,
                                    skip_group_check=True,
                                    tile_position=(0, 32 * j),
                                )
                    # compact the (g, m)-partition w to wT[k, n32] on PE
                    # (allocated in the pscg bank: both are small and their
                    # accesses are already serialized by the softmax chain)
                    wTps = psc_pool.tile([16, 32], F32, tag="wTps")
                    nc.tensor.matmul(wTps[:], w2[:], selT[:], start=True, stop=True)
                    wT = wrk.tile([16, 32], BF16, tag="wT")
                    nc.vector.tensor_copy(wT[:], wTps[:])
                    # expand w onto the (k, n8)-partition block layout: one
                    # matmul with a stride-0 16x-repeated moving operand, then
                    # a single masked multiply
                    pwx = pwx_pool.tile([128, 512], F32, tag="pwx")
                    nc.tensor.matmul(
                        pwx[:],
                        E16[:],
                        wT[:].unsqueeze(1).broadcast_to([16, 16, 32]),
                        start=True,
                        stop=True,
                    )
                    masked = wrk.tile([128, 512], BF16, tag="masked")
                    nc.vector.tensor_mul(masked[:], pwx[:], M32R[:])
                    for r in range(2):
                        for g in range(G):
                            for j in range(4):
                                nc.tensor.matmul(
                                    strips[32 * j : 32 * (j + 1),
                                           512 * r : 512 * (r + 1)],
                                    masked[:, g * 128 + 32 * j : g * 128 + 32 * (j + 1)],
                                    P_sb[:, g * H4 + j * 1024 + r * 512 : g * H4 + j * 1024 + r * 512 + 512],
                                    start=False,
                                    stop=(g == G - 1),
                                    skip_group_check=True,
                                    tile_position=(0, 32 * j),
                                )
                    # PE filler for the cast window
                    fill(2)

                    # -- one cast, 8 transposes into one bf16 PSUM bank,
                    # one xproj add, strided activations, cell update
                    sg = wrk.tile([128, 1024], BF16, tag="sg")
                    nc.scalar.copy(sg[:], strips[:])
                    pat = paT_pool.tile([128, 1024], BF16, tag="pat",
                                        name=f"pat_{t}")
                    for m in range(8):
                        nc.tensor.matmul(
                            pat[:, 128 * m : 128 * (m + 1)],
                            sg[:, 128 * m : 128 * (m + 1)],
                            eye[:],
                            is_transpose=True,
                            start=(m == 0),
                            stop=(m == 7),
                        )
                    # PE filler for the cell-math window
                    fill(4)

                    ssum = wrk.tile([128, 1024], BF16, tag="ssum")
                    nc.vector.tensor_add(ssum[:], pat[:], xptf[:])
                    act = wrk.tile([128, 1024], F32, tag="act")
                    nc.scalar.activation(
                        mv(act[:])[:, :, 0:96], mv(ssum[:])[:, :, 0:96], AF.Sigmoid
                    )
                    nc.scalar.activation(
                        mv(act[:])[:, :, 96:128], mv(ssum[:])[:, :, 96:128], AF.Tanh
                    )
                    warm(2, act[:, 0:16])
                    i_v = mv(act[:])[:, :, 0:32]
                    f_v = mv(act[:])[:, :, 32:64]
                    o_v = mv(act[:])[:, :, 64:96]
                    g_v = mv(act[:])[:, :, 96:128]
                    ig = wrk.tile([128, 256], F32, tag="ig")
                    nc.vector.tensor_mul(mn(ig[:]), i_v, g_v)
                    fc = wrk.tile([128, 256], F32, tag="fc")
                    nc.vector.tensor_mul(mn(fc[:]), f_v, mn(cT[:]))
                    nc.vector.tensor_add(cT[:], ig[:], fc[:])
                    tch = wrk.tile([128, 256], F32, tag="tch")
                    nc.scalar.activation(tch[:], cT[:], AF.Tanh)
                    warm(3, tch[:, 0:16])
                    h32all = wrk.tile([128, 256], F32, tag="h32", name=f"h32_{t}")
                    nc.vector.tensor_mul(mn(h32all[:]), o_v, mn(tch[:]))
                    # write h into uTh (bf16) for step t+1
                    nc.vector.tensor_copy(uTh[:], h32all[:])
                    nc.gpsimd.dma_start(
                        outT[t].rearrange("(r q p) n -> p r q n", r=2, p=128),
                        h32all[:].rearrange("p (r q n) -> p r q n", r=2, q=4),
                    )
                    if warm_on:
                        # liveness read so DCE keeps the warm-keeper matmuls
                        junk = wrk.tile([16, 32], F32, tag="junk",
                                        name=f"junk_{t}")
                        nc.vector.tensor_copy(junk[:], pscg[0:16, 32:64])
                    # deferred chunk evacuations: run on ACT/DVE idle time
                    # during the next step's scores/gates region
                    for W_, tb_, psx_ in pend:
                        xproj_evac(W_, tb_, psx_)
                while ci < len(chunks):
                    xproj_chunk(*chunks[ci])
                    ci += 1
            _ax.close()
    nc.compile()
    return nc


def _prep_shards(inputs):
    x = np.asarray(inputs["x"], np.float32)
    A = np.asarray(inputs["A"], np.float32)
    Wx = np.asarray(inputs["Wx"], np.float32)
    Wh = np.asarray(inputs["Wh"], np.float32)
    Wattn = np.asarray(inputs["Wattn"], np.float32)
    b = np.asarray(inputs["b"], np.float32)

    Wx_bf = np.ascontiguousarray(Wx.astype(BF))
    Wh_bf = np.ascontiguousarray(Wh.astype(BF))
    Wa_bf = np.ascontiguousarray(Wattn.astype(BF))
    bT = np.ascontiguousarray(b.reshape(32, 128).T.astype(np.float32))

    in_maps = []
    for i in range(NCORES):
        ns = slice(NL * i, NL * (i + 1))
        xT = x[ns].transpose(2, 1, 0).reshape(D, T * NL)
        Asc = A[ns].reshape(NL, H, 16).transpose(1, 2, 0).reshape(H, 512)
        in_maps.append(
            {
                "xT": np.ascontiguousarray(xT.astype(BF)),
                "Asc": np.ascontiguousarray(Asc.astype(BF)),
                "Asc32": np.ascontiguousarray(Asc.astype(np.float32)),
                "Wx": Wx_bf,
                "Wh": Wh_bf,
                "Wattn": Wa_bf,
                "bT": bT,
            }
        )
    return in_maps


def _get_nc():
    global _built
    if _built is None:
        _built = _build_nc()
    return _built


def _run(inputs, **kwargs):
    nc = _get_nc()
    in_maps = _prep_shards(inputs)
    res = bass_utils.run_bass_kernel_spmd(
        nc, in_maps, core_ids=list(range(NCORES)), **kwargs
    )
    out = np.empty((N, T, H), np.float32)
    for i in range(NCORES):
        out[NL * i : NL * (i + 1)] = res.results[i]["outT"].transpose(2, 0, 1)
    return out, res


def kernel(**inputs):
    out, _ = _run(inputs)
    return out


# revision 25
# speedup vs baseline: 1.2469x; 1.2469x over previous
"""Trainium2 Bass kernel for nn_CaptioningRNN (attention LSTM over T=64).

Data-parallel over the batch: N=256 samples split across 8 NeuronCores
(32 samples/core), weights replicated, no collectives.

Per-core algorithm (all matmuls bf16 on the TensorEngine, state in f32):
  1. xproj: xpT = (x @ Wx + b) computed transposed via Wx-stationary
     matmuls into a DRAM scratch.  Time is split into 8 blocks of 8
     steps; block 0 runs up front, blocks 1-7 are emitted as PE filler
     inside the recurrence (4 chunks per step, steps 0-55) at the three
     points where the serial softmax/cast/cell chains would otherwise
     idle the PE and let HAM re-throttle the clock.
  2. P phase: P[n, k, :] = A[n, :, k] @ Wattn precomputed once (the
     attention context contribution to the gates becomes a w-weighted
     sum of P rows).  h0 = c0 = mean_k(A) from an f32 copy of A.
  3. Recurrence (64 steps):
     - scores via hT-chunk matmuls against a permuted A (cross-sample
       products in PSUM, diagonal extracted with a mask+reduce on DVE)
     - softmax on [32,16] via the sigmoid table (no max-subtract: the
       scores are O(1) by construction), e^s = y/(1-y)
     - w compacted/expanded on PE to a block-diagonal stationary
     - gates = h @ Wh + sum_k w_k P_k accumulated into ONE [128,1024]
       PSUM strip pair using 4-way tensor-engine column tiling
     - one cast, 8 PE transposes into one bf16 PSUM bank, one xproj
       add, strided activations, cell math on [128,256] views
  4. Output written transposed [t, h, n]; host reassembles to (N, T, H).
"""

from contextlib import ExitStack

import numpy as np
import ml_dtypes

import concourse.bacc as bacc
import concourse.mybir as mybir
from concourse import bass_utils
from concourse.tile import TileContext, add_dep_helper

F32, BF16 = mybir.dt.float32, mybir.dt.bfloat16
AF = mybir.ActivationFunctionType
ALU = mybir.AluOpType
AX = mybir.AxisListType
BF = ml_dtypes.bfloat16

N, T, D, H = 256, 64, 1024, 1024
NCORES = 8
NL = N // NCORES          # 32 samples per core
HC = 8                    # 128-row chunks of D/H
G, GS = 4, 8              # sample groups of 8 (for the (k, n_g) 128-partition layout)
H4 = 4 * H                # 4096 gate columns
TB = 8                    # xproj time-block length (steps per chunk)

_built = None


def _consts():
    # E16[k', 8k + n] = (k' == k): one-hot expansion of wT rows onto the
    # (k-major, n_g-minor) 128-partition layout.
    e16 = np.zeros((16, 128), dtype=BF)
    for k in range(16):
        e16[k, 8 * k : 8 * k + 8] = 1
    # M32R[p, 128 g + 32 rep + m] = (m % 8 == p % 8) & (m // 8 == g):
    # block-diagonal mask producing masked_g = w[m, k(p)] only for group-g
    # samples, replicated 4x for the column-tiled matmuls.
    p = np.arange(128)[:, None]
    m = np.arange(32)[None, :]
    m32r = np.zeros((128, 512), dtype=BF)
    for g in range(4):
        blk = ((m % 8 == p % 8) & (m // 8 == g)).astype(BF)
        for rep in range(4):
            m32r[:, 128 * g + 32 * rep : 128 * g + 32 * rep + 32] = blk
    # Mdiag8[32 g + m, 8 k + n] = (m == 8 g + n) / 32: extracts the
    # group-local diagonal of the score products (stationary = all 32
    # samples, moving = group-g A columns) and applies the 1/sqrt(H) scale.
    md8 = np.zeros((128, 128), dtype=np.float32)
    for g in range(4):
        for n in range(8):
            for k in range(16):
                md8[32 * g + 8 * g + n, 8 * k + n] = 1.0 / 32.0
    # selT[32 g + (8 g + n), 8 g + n] = 1: compacts the block-diagonal w
    # layout to wT[k, n] via a single PE matmul (stationary = w2).
    sel = np.zeros((128, 32), dtype=BF)
    for g in range(4):
        for n in range(8):
            sel[32 * g + 8 * g + n, 8 * g + n] = 1
    return e16, m32r, md8, sel


def _build_nc(t_steps=T):
    nc = bacc.Bacc(trn_type="TRN2", target_bir_lowering=False, debug=False)

    ap_xT = nc.dram_tensor("xT", [D, T * NL], BF16, kind="ExternalInput").ap()
    ap_Asc = nc.dram_tensor("Asc", [H, 512], BF16, kind="ExternalInput").ap()
    ap_Asc32 = nc.dram_tensor("Asc32", [H, 512], F32, kind="ExternalInput").ap()
    ap_Wx = nc.dram_tensor("Wx", [D, H4], BF16, kind="ExternalInput").ap()
    ap_Wh = nc.dram_tensor("Wh", [H, H4], BF16, kind="ExternalInput").ap()
    ap_Wattn = nc.dram_tensor("Wattn", [H, H4], BF16, kind="ExternalInput").ap()
    ap_bT = nc.dram_tensor("bT", [128, 32], F32, kind="ExternalInput").ap()
    outT = nc.dram_tensor("outT", [T, H, NL], F32, kind="ExternalOutput").ap()
    # xps[r, t, q, j, p, n] = xproj[t][n, j*1024 + r*512 + q*128 + p]
    # (t outermost-after-r so the per-step load is one contiguous 64 KiB
    # block per r; the chunk stores scatter over t but have deep slack)
    xps = nc.dram_tensor("xps", [2, T, 4, 4, 128, NL], BF16, kind="Internal").ap()

    e16_np, m32r_np, md8_np, sel_np = _consts()
    eye_d = nc.inline_tensor(np.eye(128, dtype=BF), "c_eye")
    e16_d = nc.inline_tensor(e16_np, "c_e16")
    m32r_d = nc.inline_tensor(m32r_np, "c_m32r")
    md8_d = nc.inline_tensor(md8_np, "c_mdiag8")
    sel_d = nc.inline_tensor(sel_np, "c_selT")

    with TileContext(nc) as tc:
        with tc.tile_pool(name="pers", bufs=1) as pers:
            Wh_sb = pers.tile([128, HC * H4], BF16, tag="Wh")
            Asc_sb = pers.tile([128, HC * 512], BF16, tag="Asc")
            P_sb = pers.tile([128, G * H4], BF16, tag="P")
            uTh = pers.tile([128, HC * 32], BF16, tag="uTh")
            cT = pers.tile([128, 256], F32, tag="cT")
            eye = pers.tile([128, 128], BF16, tag="eye")
            E16 = pers.tile([16, 128], BF16, tag="E16")
            M32R = pers.tile([128, 512], BF16, tag="M32R")
            Mdiag8 = pers.tile([128, 128], F32, tag="Mdiag8")
            selT = pers.tile([128, 32], BF16, tag="selT")
            b_sb = pers.tile([128, 32], F32, tag="bT")
            Ag = pers.tile([128, G * HC * 128], BF16, tag="Ag")

            nc.sync.dma_start(eye[:], eye_d.ap()[:])
            nc.sync.dma_start(E16[:], e16_d.ap()[:])
            nc.sync.dma_start(M32R[:], m32r_d.ap()[:])
            nc.sync.dma_start(Mdiag8[:], md8_d.ap()[:])
            nc.sync.dma_start(selT[:], sel_d.ap()[:])
            nc.sync.dma_start(b_sb[:], ap_bT[:])
            nc.sync.dma_start(
                Wh_sb[:].rearrange("p (c x) -> p c x", c=HC),
                ap_Wh.rearrange("(c p) x -> p c x", p=128),
            )
            nc.sync.dma_start(
                Asc_sb[:].rearrange("p (c x) -> p c x", c=HC),
                ap_Asc.rearrange("(c p) x -> p c x", p=128),
            )

            # ---------------- phase A: xproj -> DRAM scratch ----------------
            # Pools stay open through the recurrence so block>=1 chunks can
            # be interleaved between steps (fills PE-idle gaps, keeps HAM
            # warm).
            _ax = ExitStack()
            phx1 = _ax.enter_context(tc.tile_pool(name="phx1", bufs=1))
            phxW = _ax.enter_context(tc.tile_pool(name="phxW", bufs=8))
            phxS = _ax.enter_context(tc.tile_pool(name="phxS", bufs=6))
            psX = _ax.enter_context(tc.tile_pool(name="psX", bufs=2, space="PSUM"))
            xT_sb = phx1.tile([128, HC * T * NL], BF16, tag="xTsb")
            # split the 4 MB load so the first chunks can start early
            for c in range(HC):
                nc.sync.dma_start(
                    xT_sb[:, c * T * NL : (c + 1) * T * NL],
                    ap_xT.rearrange("(c p) x -> p c x", p=128)[:, c],
                )

            nchunk = [0]
            # xps store DMA instructions per time-block, so the per-step
            # xpt loads can take an explicit dependency on them (the DRAM
            # scratch is not covered by tile overlap tracking)
            xps_stores = [[] for _ in range(T // TB)]

            def xproj_mm(W, tb):
                j, r, q = W // 8, (W % 8) // 4, W % 4
                t0, t1 = TB * tb, TB * (tb + 1)
                nt = (t1 - t0) * NL
                Wxb = phxW.tile(
                    [128, HC * 128], BF16, tag="Wxb", name=f"Wxb_{W}_{tb}"
                )
                nc.sync.dma_start(
                    Wxb[:].rearrange("p (c x) -> p c x", c=HC),
                    ap_Wx.rearrange("(c p) x -> p c x", p=128)[
                        :, :, 128 * W : 128 * (W + 1)
                    ],
                )
                psx = psX.tile([128, nt], F32, tag="psx", name=f"psx_{W}_{tb}")
                for c in range(HC):
                    nc.tensor.matmul(
                        psx[:],
                        Wxb[:, c * 128 : (c + 1) * 128],
                        xT_sb[:, c * T * NL + NL * t0 : c * T * NL + NL * t1],
                        start=(c == 0),
                        stop=(c == HC - 1),
                    )
                return psx

            def xproj_evac(W, tb, psx):
                j, r, q = W // 8, (W % 8) // 4, W % 4
                t0, t1 = TB * tb, TB * (tb + 1)
                nt = (t1 - t0) * NL
                ci = nchunk[0]
                nchunk[0] += 1
                sxp = phxS.tile([128, nt], BF16, tag="sxp", name=f"sxp_{W}_{tb}")
                # alternate the PSUM evacuation engine so chunk fills don't
                # serialize behind the step's ACT/DVE chain work
                if ci % 2 == 0:
                    nc.scalar.add(sxp[:], psx[:], b_sb[:, W : W + 1])
                else:
                    nc.vector.tensor_scalar_add(sxp[:], psx[:], b_sb[:, W : W + 1])
                # store on the gpsimd queue so the Wxb prefetch stream
                # (sync queue) is never stuck behind these scatters
                st = nc.gpsimd.dma_start(
                    xps[r, t0:t1, q, j].transpose([1, 0, 2]),
                    sxp[:].rearrange("p (t n) -> p t n", t=t1 - t0),
                )
                xps_stores[tb].append(st.ins)

            def xproj_chunk(W, tb):
                xproj_evac(W, tb, xproj_mm(W, tb))

            # block 0 up front (also serves as the HAM warm-up ramp)
            for W in range(32):
                xproj_chunk(W, 0)

            # ------------- phase B: P precompute + h0/c0 init -------------
            with tc.tile_pool(name="php1", bufs=1) as php1, \
                 tc.tile_pool(name="php", bufs=3) as php, \
                 tc.tile_pool(name="psP", bufs=2, space="PSUM") as psP:
                A32 = php1.tile([128, HC * 512], F32, tag="A32")
                nc.sync.dma_start(
                    A32[:].rearrange("p (c x) -> p c x", c=HC),
                    ap_Asc32.rearrange("(c p) x -> p c x", p=128),
                )
                for c in range(HC):
                    h0s = php.tile([128, 32], F32, tag="h0s")
                    nc.vector.tensor_reduce(
                        h0s[:],
                        A32[:, c * 512 : (c + 1) * 512].rearrange(
                            "p (k n) -> p n k", k=16
                        ),
                        axis=AX.X,
                        op=ALU.add,
                    )
                    nc.vector.tensor_scalar_mul(
                        cT[:, 32 * c : 32 * (c + 1)], h0s[:], 1.0 / 16.0
                    )
                    nc.vector.tensor_copy(
                        uTh[:, 32 * c : 32 * (c + 1)],
                        cT[:, 32 * c : 32 * (c + 1)],
                    )
                # contiguous staging of the group-selected A columns so the
                # matmul stationary operand has a single free dim
                for g in range(G):
                    for c in range(HC):
                        nc.vector.tensor_copy(
                            Ag[:, (g * HC + c) * 128 : (g * HC + c) * 128 + 128],
                            Asc_sb[:, c * 512 : (c + 1) * 512].rearrange(
                                "p (k n) -> p k n", k=16
                            )[:, :, GS * g : GS * (g + 1)],
                        )
                for blk in range(8):
                    Wab = php.tile([128, HC * 512], BF16, tag="Wab")
                    nc.sync.dma_start(
                        Wab[:].rearrange("p (c x) -> p c x", c=HC),
                        ap_Wattn.rearrange("(c p) x -> p c x", p=128)[
                            :, :, 512 * blk : 512 * (blk + 1)
                        ],
                    )
                    for g in range(G):
                        psp = psP.tile([128, 512], F32, tag="psp")
                        for c in range(HC):
                            nc.tensor.matmul(
                                psp[:],
                                Ag[:, (g * HC + c) * 128 : (g * HC + c) * 128 + 128],
                                Wab[:, c * 512 : (c + 1) * 512],
                                start=(c == 0),
                                stop=(c == HC - 1),
                            )
                        nc.vector.tensor_copy(
                            P_sb[:, g * H4 + 512 * blk : g * H4 + 512 * (blk + 1)],
                            psp[:],
                        )

            # ---------------------- phase C: recurrence ----------------------
            with tc.tile_pool(name="wrk", bufs=2) as wrk, \
                 tc.tile_pool(name="psc", bufs=1, space="PSUM") as psc_pool, \
                 tc.tile_pool(name="pwx", bufs=1, space="PSUM") as pwx_pool, \
                 tc.tile_pool(name="pstr", bufs=1, space="PSUM") as pstr_pool, \
                 tc.tile_pool(name="paT", bufs=1, space="PSUM") as paT_pool:
                # deferred xproj chunks in deadline order: block tb is
                # consumed during steps [8*(tb-1), 8*tb) and must land
                # before step 8*tb reads it (Tile semaphores enforce it).
                chunks = [
                    (W, tb)
                    for tb in range(1, T // TB)
                    if TB * tb < t_steps
                    for W in range(32)
                ]
                ci = 0
                mv = lambda ap: ap.rearrange("p (m x) -> p m x", m=8)
                mn = lambda ap: ap.rearrange("p (m n) -> p m n", m=8)
                for t in range(t_steps):
                    # chunk matmuls are emitted in the tail stall windows;
                    # their PSUM evacuations are deferred to the end of the
                    # step so they land on ACT/DVE idle time during the next
                    # step's scores/gates instead of inside the serial chain
                    pend = []

                    def fill(k):
                        nonlocal ci
                        want = min(len(chunks), 4 * (t + 1) + k)
                        while ci < want:
                            W_, tb_ = chunks[ci]
                            pend.append((W_, tb_, xproj_mm(W_, tb_)))
                            ci += 1

                    # prefetched xproj slice for this step, laid out
                    # (r, q, j, n) to match the transposed strip
                    xptf = wrk.tile([128, 1024], BF16, tag="xpt", name=f"xpt_{t}")
                    for r in range(2):
                        ld = nc.gpsimd.dma_start(
                            xptf[:, 512 * r : 512 * (r + 1)].rearrange(
                                "p (c n) -> p c n", c=16
                            ),
                            xps[r, t].rearrange("q j p n -> p (q j) n"),
                        )
                        for st_ins in xps_stores[t // TB]:
                            add_dep_helper(
                                ld.ins, st_ins, reason="xps block store->load"
                            )

                    # -- scores: per-group (8-sample) products against Ag with
                    # 4-way col tiling, group-local diag extract, softmax
                    pscg = psc_pool.tile([128, 128], F32, tag="psc")
                    for c in range(HC):
                        for g in range(G):
                            nc.tensor.matmul(
                                pscg[32 * g : 32 * (g + 1), :],
                                uTh[:, c * 32 : (c + 1) * 32],
                                Ag[:, (g * HC + c) * 128 : (g * HC + c + 1) * 128],
                                start=(c == 0),
                                stop=(c == HC - 1),
                                skip_group_check=True,
                                tile_position=(0, 32 * g),
                            )
                    scm = wrk.tile([128, 128], F32, tag="scm")
                    nc.vector.tensor_mul(scm[:], pscg[:], Mdiag8[:])
                    scores = wrk.tile([128, 16], F32, tag="scores")
                    nc.vector.tensor_reduce(
                        scores[:],
                        scm[:].rearrange("p (k n) -> p k n", k=16),
                        axis=AX.X,
                        op=ALU.add,
                    )
                    # softmax via the sigmoid table (keeps every ACT op in the
                    # sigmoid_and_others set -> one table load for the kernel):
                    # y = sigmoid(s), e^s = y / (1 - y); the scores are O(1)
                    # by construction so instead of a max-subtraction a clamp
                    # at 12 guards the y -> 1 division (a no-op in practice)
                    scl = wrk.tile([128, 16], F32, tag="scl")
                    nc.vector.tensor_scalar_min(scl[:], scores[:], 12.0)
                    ysig = wrk.tile([128, 16], F32, tag="ysig")
                    nc.scalar.activation(ysig[:], scl[:], AF.Sigmoid)

                    # warm-keepers: only for the filler-less tail steps
                    warm_on = ci >= len(chunks) and t < t_steps - 1

                    def warm(i, dep):
                        if warm_on:
                            nc.tensor.matmul(
                                pscg[0:16, 16 * i : 16 * (i + 1)],
                                dep,
                                dep,
                                start=True,
                                stop=True,
                            )

                    omy = wrk.tile([128, 16], F32, tag="omy")
                    nc.vector.tensor_scalar(
                        omy[:], ysig[:], -1.0, 1.0, ALU.mult, ALU.add
                    )
                    romy = wrk.tile([128, 16], F32, tag="romy")
                    nc.vector.reciprocal(romy[:], omy[:])
                    ex = wrk.tile([128, 16], F32, tag="ex")
                    esum = wrk.tile([128, 1], F32, tag="esum")
                    nc.vector.scalar_tensor_tensor(
                        ex[:], ysig[:], 1.0, romy[:], ALU.mult, ALU.mult,
                        accum_out=esum[:],
                    )
                    rcp = wrk.tile([128, 1], F32, tag="rcp")
                    nc.vector.reciprocal(rcp[:], esum[:])
                    w2 = wrk.tile([128, 16], BF16, tag="w2")
                    nc.vector.tensor_scalar_mul(w2[:], ex[:], rcp[:])

                    # -- gates: h @ Wh + sum_k w_k P_k into one column-tiled
                    # [128, 1024] strip pair (r = 512-col halves).  The Wh
                    # matmuls are emitted BEFORE the w-expand PE ops so the
                    # softmax chain hides behind ~4.4us of scores+Wh instead
                    # of head-of-line-blocking the PE queue at wTps.
                    strips = pstr_pool.tile([128, 1024], F32, tag="strips",
                                            name=f"strips_{t}")
                    for c in range(HC):
                        for r in range(2):
                            for j in range(4):
                                nc.tensor.matmul(
                                    strips[32 * j : 32 * (j + 1),
                                           512 * r : 512 * (r + 1)],
                                    uTh[:, c * 32 : (c + 1) * 32],
                                    Wh_sb[:, c * H4 + j * 1024 + r * 512 : c * H4 + j * 1024 + r * 512 + 512],
                                    start=(c == 0),
                                    stop=False,
                                    skip_group_check=True,
                                    tile_position=(0, 32 * j),
                                )
                    # compact the (g, m)-partition w to wT[k, n32] on PE
                    # (allocated in the pscg bank: both are small and their
                    # accesses are already serialized by the softmax chain)
                    wTps = psc_pool.tile([16, 32], F32, tag="wTps")
                    nc.tensor.matmul(wTps[:], w2[:], selT[:], start=True, stop=True)
                    wT = wrk.tile([16, 32], BF16, tag="wT")
                    nc.vector.tensor_copy(wT[:], wTps[:])
                    # expand w onto the (k, n8)-partition block layout: one
                    # matmul with a stride-0 16x-repeated moving operand, then
                    # a single masked multiply
                    pwx = pwx_pool.tile([128, 512], F32, tag="pwx")
                    nc.tensor.matmul(
                        pwx[:],
                        E16[:],
                        wT[:].unsqueeze(1).broadcast_to([16, 16, 32]),
                        start=True,
                        stop=True,
                    )
                    masked = wrk.tile([128, 512], BF16, tag="masked")
                    nc.vector.tensor_mul(masked[:], pwx[:], M32R[:])
                    for r in range(2):
                        for g in range(G):
                            for j in range(4):
                                nc.tensor.matmul(
                                    strips[32 * j : 32 * (j + 1),
                                           512 * r : 512 * (r + 1)],
                                    masked[:, g * 128 + 32 * j : g * 128 + 32 * (j + 1)],
                                    P_sb[:, g * H4 + j * 1024 + r * 512 : g * H4 + j * 1024 + r * 512 + 512],
                                    start=False,
                                    stop=(g == G - 1),
                                    skip_group_check=True,
                                    tile_position=(0, 32 * j),
                                )
                    # PE filler for the cast window
                    fill(2)

                    # -- one cast, 8 transposes into one bf16 PSUM bank,
                    # one xproj add, strided activations, cell update
                    sg = wrk.tile([128, 1024], BF16, tag="sg")
                    nc.scalar.copy(sg[:], strips[:])
                    pat = paT_pool.tile([128, 1024], BF16, tag="pat",
                                        name=f"pat_{t}")
                    for m in range(8):
                        nc.tensor.matmul(
                            pat[:, 128 * m : 128 * (m + 1)],
                            sg[:, 128 * m : 128 * (m + 1)],
                            eye[:],
                            is_transpose=True,
                            start=(m == 0),
                            stop=(m == 7),
                        )
                    # PE filler for the cell-math window
                    fill(4)

                    ssum = wrk.tile([128, 1024], BF16, tag="ssum")
                    nc.vector.tensor_add(ssum[:], pat[:], xptf[:])
                    act = wrk.tile([128, 1024], F32, tag="act")
                    nc.scalar.activation(
                        mv(act[:])[:, :, 0:96], mv(ssum[:])[:, :, 0:96], AF.Sigmoid
                    )
                    nc.scalar.activation(
                        mv(act[:])[:, :, 96:128], mv(ssum[:])[:, :, 96:128], AF.Tanh
                    )
                    warm(2, act[:, 0:16])
                    i_v = mv(act[:])[:, :, 0:32]
                    f_v = mv(act[:])[:, :, 32:64]
                    o_v = mv(act[:])[:, :, 64:96]
                    g_v = mv(act[:])[:, :, 96:128]
                    ig = wrk.tile([128, 256], F32, tag="ig")
                    nc.vector.tensor_mul(mn(ig[:]), i_v, g_v)
                    fc = wrk.tile([128, 256], F32, tag="fc")
                    nc.vector.tensor_mul(mn(fc[:]), f_v, mn(cT[:]))
                    nc.vector.tensor_add(cT[:], ig[:], fc[:])
                    tch = wrk.tile([128, 256], F32, tag="tch")
                    nc.scalar.activation(tch[:], cT[:], AF.Tanh)
                    warm(3, tch[:, 0:16])
                    h32all = wrk.tile([128, 256], F32, tag="h32", name=f"h32_{t}")
                    nc.vector.tensor_mul(mn(h32all[:]), o_v, mn(tch[:]))
                    # write h into uTh (bf16) for step t+1
                    nc.vector.tensor_copy(uTh[:], h32all[:])
                    nc.gpsimd.dma_start(
                        outT[t].rearrange("(r q p) n -> p r q n", r=2, p=128),
                        h32all[:].rearrange("p (r q n) -> p r q n", r=2, q=4),
                    )
                    if warm_on:
                        # liveness read so DCE keeps the warm-keeper matmuls
                        junk = wrk.tile([16, 32], F32, tag="junk",
                                        name=f"junk_{t}")
                        nc.vector.tensor_copy(junk[:], pscg[0:16, 32:64])
                    # deferred chunk evacuations: run on ACT/DVE idle time
                    # during the next step's scores/gates region
                    for W_, tb_, psx_ in pend:
                        xproj_evac(W_, tb_, psx_)
                while ci < len(chunks):
                    xproj_chunk(*chunks[ci])
                    ci += 1
            _ax.close()
    nc.compile()
    return nc


def _prep_shards(inputs):
    x = np.asarray(inputs["x"], np.float32)
    A = np.asarray(inputs["A"], np.float32)
    Wx = np.asarray(inputs["Wx"], np.float32)
    Wh = np.asarray(inputs["Wh"], np.float32)
    Wattn = np.asarray(inputs["Wattn"], np.float32)
    b = np.asarray(inputs["b"], np.float32)

    Wx_bf = np.ascontiguousarray(Wx.astype(BF))
    Wh_bf = np.ascontiguousarray(Wh.astype(BF))
    Wa_bf = np.ascontiguousarray(Wattn.astype(BF))
    bT = np.ascontiguousarray(b.reshape(32, 128).T.astype(np.float32))

    in_maps = []
    for i in range(NCORES):
        ns = slice(NL * i, NL * (i + 1))
        xT = x[ns].transpose(2, 1, 0).reshape(D, T * NL)
        Asc = A[ns].reshape(NL, H, 16).transpose(1, 2, 0).reshape(H, 512)
        in_maps.append(
            {
                "xT": np.ascontiguousarray(xT.astype(BF)),
                "Asc": np.ascontiguousarray(Asc.astype(BF)),
                "Asc32": np.ascontiguousarray(Asc.astype(np.float32)),
                "Wx": Wx_bf,
                "Wh": Wh_bf,
                "Wattn": Wa_bf,
                "bT": bT,
            }
        )
    return in_maps


def _get_nc():
    global _built
    if _built is None:
        _built = _build_nc()
    return _built


def _run(inputs, **kwargs):
    nc = _get_nc()
    in_maps = _prep_shards(inputs)
    res = bass_utils.run_bass_kernel_spmd(
        nc, in_maps, core_ids=list(range(NCORES)), **kwargs
    )
    out = np.empty((N, T, H), np.float32)
    for i in range(NCORES):
        out[NL * i : NL * (i + 1)] = res.results[i]["outT"].transpose(2, 0, 1)
    return out, res


def kernel(**inputs):
    out, _ = _run(inputs)
    return out
